# revision 7
# baseline (speedup 1.0000x reference)
"""GCN2 layer (message passing + initial residual + 64x64 linear + relu)
on 8 Trainium2 NeuronCores via Bass/Tile.

Strategy (graph/data parallel, destination-sharded):
  - Normalization folded on host: deg/dinv via bincount; per-edge weight
    wm = 0.9*dinv[row]*w*dinv[col]; per-node residual scale
    s = 0.9*dinv^2 + 0.1 shipped as a per-window diagonal matrix.
  - x replicated to every core host-side (the halo exchange), stored
    bf16 with rows padded to 256B so dma_gather can fetch them.
  - Pad N=100000 -> 100352 = 8 shards x 12544 (98 windows of 128).
  - Message phase: edges sharded by DEST core, grouped by (dest-window,
    source-bank); dma_gather fetches x[col] rows (int16 idx per
    25088-row bank, trailing -1 idx = skipped padding); one-hot
    scatter-matmuls (transposed: aggT = msgsT @ onehot) accumulate
    [C, 128] tiles in PSUM; +1 matmul vs a diagonal adds the
    self-loop/residual term.  PSUM copied into a resident hT buffer.
  - Epilogue: outT = relu(W1^T @ hT) in 512-wide batched fp32 matmuls,
    single 3.2MB output DMA; host transposes/unpads.
"""

import numpy as np
import ml_dtypes

N, E, C, M = 100000, 1200000, 64, 8
NL, WIN = 12500, 128
NW = 98
NLP = NW * WIN            # 12544 padded nodes per core
NP = M * NLP              # 100352 padded total
NBANK = 4
BANK = NP // NBANK        # 25088 rows per gather bank (int16-safe)
NCELL = NW * NBANK
XROW = 128                # bf16 row stride of replicated x (256B)
BF16 = ml_dtypes.bfloat16

_CACHE = {}


def _host_prep(x, edge_index, edge_weight):
    row = np.asarray(edge_index[0], dtype=np.int64)   # dest
    col = np.asarray(edge_index[1], dtype=np.int64)   # src
    w = np.asarray(edge_weight, dtype=np.float32)
    x = np.asarray(x, dtype=np.float32)

    deg = np.bincount(col, weights=w, minlength=N).astype(np.float32) + 1.0
    dinv = 1.0 / np.sqrt(deg)
    wme = (0.9 * dinv[row] * w * dinv[col]).astype(np.float32)
    s = 0.9 * dinv * dinv + 0.1                       # self-loop + residual

    dcore = row // NL
    rl = row % NL
    mwin = rl // WIN
    rloc = (rl % WIN).astype(np.float32)
    gcol = (col // NL) * NLP + (col % NL)
    bank = gcol // BANK
    idx16 = (gcol - bank * BANK).astype(np.int16)
    cell = mwin * NBANK + bank

    saved = []
    maxcnt = 0
    for m in range(M):
        sel = np.nonzero(dcore == m)[0]
        c = cell[sel]
        order = np.argsort(c, kind="stable")
        eidx = sel[order]
        cs = c[order]
        cnt = np.bincount(cs, minlength=NCELL)
        maxcnt = max(maxcnt, int(cnt.max()))
        saved.append((eidx, cs, cnt))
    S = max(1, -(-maxcnt // WIN))
    CELLE = S * WIN
    # per-cell gather length, equal on every core (the num_idxs_reg
    # immediate is baked into the SPMD program): max count across cores
    gmax = np.maximum.reduce([cnt for _, _, cnt in saved]).astype(np.int64)

    pos_grid = np.arange(CELLE)[None, :]
    msg_in = []
    for m in range(M):
        eidx, cs, cnt = saved[m]
        starts = np.concatenate([[0], np.cumsum(cnt)[:-1]])
        pos = np.arange(len(cs)) - starts[cs]
        flat = cs * CELLE + pos
        idx_m = np.full(NCELL * CELLE, -1, np.int16)
        w_m = np.zeros(NCELL * CELLE, np.float32)
        rl_m = np.zeros(NCELL * CELLE, np.float32)
        idx_m[flat] = idx16[eidx]
        w_m[flat] = wme[eidx]
        rl_m[flat] = rloc[eidx]
        # pad [cnt, gmax) with index 0 (gathered, killed by wm=0) so the
        # Q7 trim lands exactly on gmax = num_idxs_reg on every core
        fill = (pos_grid >= cnt[:, None]) & (pos_grid < gmax[:, None])
        idx_m.reshape(NCELL, CELLE)[fill] = 0
        idxm = np.tile(
            idx_m.reshape(NCELL, CELLE // 16, 16).transpose(2, 0, 1).reshape(16, -1),
            (8, 1)).copy()
        # [pos, win, bank, slot] layout for wm/rl tables
        wm = w_m.reshape(NW, NBANK, S, WIN).transpose(3, 0, 1, 2).reshape(
            WIN, -1).astype(BF16).copy()
        rlm = rl_m.reshape(NW, NBANK, S, WIN).transpose(3, 0, 1, 2).reshape(
            WIN, -1).astype(BF16).copy()
        msg_in.append((idxm, wm, rlm))

    # replicated, padded, bf16 x: [NP, 128] rows (first 64 = features)
    xg = np.zeros((NP, XROW), BF16)
    xs3 = x.reshape(M, NL, C)
    for m in range(M):
        xg[m * NLP:m * NLP + NL, :C] = xs3[m]

    xr_in, dg_in = [], []
    s3 = s.reshape(M, NL)
    for m in range(M):
        xp = np.zeros((NLP, C), np.float32)
        xp[:NL] = xs3[m]
        xr = xp.reshape(NW, WIN, C).transpose(1, 0, 2).reshape(WIN, -1)
        xr_in.append(xr.astype(BF16).copy())
        sp = np.zeros(NLP, np.float32)
        sp[:NL] = s3[m]
        dgm = np.zeros((WIN, NW * WIN), np.float32)
        dgm[np.tile(np.arange(WIN), NW), np.arange(NLP)] = sp
        dg_in.append(dgm.astype(BF16).copy())

    return S, gmax, msg_in, xg, xr_in, dg_in


def _build(S, gmax, reps=1):
    from concourse import bacc, tile
    from concourse.bass import MemorySpace
    import concourse.mybir as mybir

    f32 = mybir.dt.float32
    bf16 = mybir.dt.bfloat16
    i16 = mybir.dt.int16
    eq = mybir.AluOpType.is_equal
    mult = mybir.AluOpType.mult

    CELLE = S * WIN
    NSLOT = NBANK * S

    nc = bacc.Bacc("TRN2", target_bir_lowering=False, debug=False,
                   num_devices=M, num_swdge_queues=4)

    xg_d = nc.dram_tensor("xg", [NP, XROW], bf16, kind="ExternalInput")
    xr_d = nc.dram_tensor("xr", [WIN, NW * C], bf16, kind="ExternalInput")
    dg_d = nc.dram_tensor("dg", [WIN, NW * WIN], bf16, kind="ExternalInput")
    w1_d = nc.dram_tensor("w1", [C, C], f32, kind="ExternalInput")
    wm_d = nc.dram_tensor("wm", [WIN, NW * NSLOT], bf16, kind="ExternalInput")
    rlm_d = nc.dram_tensor("rlm", [WIN, NW * NSLOT], bf16,
                           kind="ExternalInput")
    idx_d = nc.dram_tensor("idxm", [128, NCELL * CELLE // 16], i16,
                           kind="ExternalInput")
    out_d = nc.dram_tensor("out", [C, NLP], f32, kind="ExternalOutput")

    with tile.TileContext(nc) as tc:
        with (
            tc.tile_pool(name="res", bufs=1) as res,
            tc.tile_pool(name="work", bufs=3) as work,
            tc.tile_pool(name="ps", bufs=2, space=MemorySpace.PSUM) as ps,
        ):
            # resident loads
            xr_sb = res.tile([WIN, NW, C], bf16)
            nc.sync.dma_start(out=xr_sb[:], in_=xr_d.ap())
            dg_sb = res.tile([WIN, NW * WIN], bf16)
            nc.sync.dma_start(out=dg_sb[:], in_=dg_d.ap())
            w1_sb = res.tile([C, C], f32)
            nc.sync.dma_start(out=w1_sb[:], in_=w1_d.ap())
            wm_sb = res.tile([WIN, NW * NSLOT], bf16)
            nc.sync.dma_start(out=wm_sb[:], in_=wm_d.ap())
            rlm_sb = res.tile([WIN, NW * NSLOT], bf16)
            nc.sync.dma_start(out=rlm_sb[:], in_=rlm_d.ap())
            idx_sb = res.tile([128, NCELL * CELLE // 16], i16)
            nc.sync.dma_start(out=idx_sb[:], in_=idx_d.ap())

            iota_c = res.tile([WIN, NSLOT, WIN], bf16)
            nc.gpsimd.iota(iota_c[:], pattern=[[0, NSLOT], [1, WIN]], base=0,
                           channel_multiplier=0,
                           allow_small_or_imprecise_dtypes=True)

            hT = res.tile([C, NLP], f32)

            # gather buffers (manually double-buffered, primed: skipped
            # rows from -1 padding indices must not expose NaN garbage)
            mg2 = [res.tile([128, NSLOT, XROW], bf16, name=f"mg{i}",
                            tag=f"mg{i}") for i in range(2)]
            nc.vector.memset(mg2[0][:], 0.0)
            nc.vector.memset(mg2[1][:], 0.0)

            xg_ap = xg_d.ap()
            for wdw in [w for _ in range(reps) for w in range(NW)]:
                mg = mg2[wdw % 2]
                for b in range(NBANK):
                    cidx = wdw * NBANK + b
                    nc.gpsimd.dma_gather(
                        mg[:, b * S:(b + 1) * S, :],
                        xg_ap[b * BANK:(b + 1) * BANK, :],
                        idx_sb[:, cidx * (CELLE // 16):(cidx + 1) * (CELLE // 16)],
                        num_idxs=CELLE, num_idxs_reg=int(gmax[cidx]),
                        elem_size=XROW, single_packet=False, queue_num=b)
                mw = work.tile([WIN, NSLOT, C], bf16, tag="mw")
                nc.vector.tensor_tensor(
                    mw[:], mg[:, :, :C],
                    wm_sb[:, wdw * NSLOT:(wdw + 1) * NSLOT].broadcast_to(
                        [WIN, NSLOT, C]),
                    mult)
                oh = work.tile([WIN, NSLOT, WIN], bf16, tag="oh")
                nc.vector.tensor_tensor(
                    oh[:], iota_c[:],
                    rlm_sb[:, wdw * NSLOT:(wdw + 1) * NSLOT].broadcast_to(
                        [WIN, NSLOT, WIN]),
                    eq)
                aps = ps.tile([C, WIN], f32, tag="agg")
                for t in range(NSLOT):
                    nc.tensor.matmul(aps[:], mw[:, t, :], oh[:, t, :],
                                     start=(t == 0), stop=False)
                nc.tensor.matmul(aps[:], xr_sb[:, wdw, :],
                                 dg_sb[:, wdw * WIN:(wdw + 1) * WIN],
                                 start=False, stop=True)
                nc.scalar.copy(hT[:, wdw * WIN:(wdw + 1) * WIN], aps[:])

            # epilogue: outT = relu(W1^T @ hT), batched over 512 columns
            NB = 512
            for j in range(NLP // NB + (1 if NLP % NB else 0)):
                off = j * NB
                nj = min(NB, NLP - off)
                ops = ps.tile([C, NB], f32, tag="o")
                nc.tensor.matmul(ops[:, :nj], w1_sb[:], hT[:, off:off + nj],
                                 start=True, stop=True)
                nc.scalar.activation(hT[:, off:off + nj], ops[:, :nj],
                                     mybir.ActivationFunctionType.Relu)
            nc.sync.dma_start(out=out_d.ap(), in_=hT[:])

    nc.compile()
    return nc


def kernel(x, edge_index, edge_weight, W1, _reps=1):
    from concourse.bass_utils import run_bass_kernel_spmd

    S, gmax, msg_in, xg, xr_in, dg_in = _host_prep(x, edge_index, edge_weight)
    key = (S, _reps, gmax.tobytes())
    if key not in _CACHE:
        _CACHE[key] = _build(S, gmax, reps=_reps)
    nc = _CACHE[key]

    w1 = np.asarray(W1, dtype=np.float32)
    in_maps = []
    for m in range(M):
        idxm, wm, rlm = msg_in[m]
        in_maps.append({
            "xg": xg, "xr": xr_in[m], "dg": dg_in[m], "w1": w1,
            "wm": wm, "rlm": rlm, "idxm": idxm,
        })
    res = run_bass_kernel_spmd(nc, in_maps, list(range(M)))

    full = np.empty((N, C), np.float32)
    for m in range(M):
        full[m * NL:(m + 1) * NL] = res.results[m]["out"][:, :NL].T
    return full
